# revision 12
# baseline (speedup 1.0000x reference)
"""Trainium2 Bass kernel for nn_Decoder1 (linear -> BatchNorm1d -> multistep LIF).

Reference computation (T=4, B=32, N=1024, C=256):
  y[tb,o,n]   = sum_c x[tb,n,c] * W[o,c]                      (TB=128 slices)
  z           = BN(y) over (tb, n) per channel o (training stats, eps=1e-5)
  LIF over t  : v' = (v + z_t)/2 ; s = (v' >= 1) ; v' *= (1-s)
  out[tb,n',c'] = spikes[tb].reshape(C,N).T   (row-major reinterpretation)

Design (v2):
  - Data-parallel over B: 4 b-values x 4 timesteps = 16 (N,C) slices/core.
  - Matmul via fp16 3-term split: y = xh@Wh + xl@Wh + xhb@Wl (xh=fp16(x),
    xl=fp16(x-xh), xhb=bf16(xh), W* analogous, all pre-split on HOST).
    Host also pre-transposes x (c-major) so no PE transposes are needed,
    and pre-scales W by 0.5 so PSUM holds yp = y/2 directly.
  - Phase 1: hi-term only (y1 = xh@Wh'), y1 -> SBUF f32, BN stats from y1
    (bias vs full y is ~95 spike flips, rel err ~0.0095 -- measured).
  - Tiny AllReduce of (sum, sumsq); stats math in threshold units:
    v = a*h with a = gamma*rstd; spike at h >= theta = 1/a; per-step
    h' = 0.5h + yp1 + resid + d, d = 0.5*beta*theta - mean(yp1).
  - Phase 2 (post-AR, overlapped with residual matmuls): per slice
    resid = xl@Wh' + xhb@Wl' in PSUM; P1: h = 0.5h + y1 (DVE);
    P2: h = (h + d) + resid (DVE, PSUM operand); s = (h >= theta) -> u8
    (GpSimd); reset h = (h < theta)*h (DVE/GpSimd split). Spikes leave
    as uint8 (4x less DMA); host expands to f32.
"""

import numpy as np
from contextlib import ExitStack

import concourse.bass as bass
import concourse.mybir as mybir
import concourse.tile as tile

F32 = mybir.dt.float32
F16 = mybir.dt.float16
BF16 = mybir.dt.bfloat16
U8 = mybir.dt.uint8
Alu = mybir.AluOpType
ActF = mybir.ActivationFunctionType

N_CORES = 8
T, B, N, C = 4, 32, 1024, 256
B_LOC = B // N_CORES            # 4 batch entries per core
SL = T * B_LOC                  # 16 (N,C) slices per core; sl = bl*4 + t
P = 128
NS_CORE = float(SL * N)         # BN samples per channel per core
NS_TOT = float(T * B * N)       # BN samples per channel globally
BN_EPS = 1e-5

_ctr = [0]
SINGLE = False   # test-only: skip the AllReduce (single-core sim)
LEGALIZE = True  # set False for CoreSim (NoOp waits trip the race detector)
# engine tables (v=vector/DVE, g=gpsimd). GpSimd has no PSUM access and no
# scalar_tensor_tensor, so: P1/reset (stt) are DVE-only; P2 (plain tt on
# SBUF) and s (tensor_scalar AP->u8) can go to gpsimd.
S_ENG = {sl: "g" for sl in range(SL)}            # spike compare per slice
P2_ENG = {sl: ("g" if sl % 2 == 0 else "v") for sl in range(SL)}


def _legalize_waits(nc, limit=1):
    """Hoist excess semaphore waits onto same-engine NoOps (walrus accepts
    very few waits per instruction; PE matmul: 1)."""
    for f in nc.m.functions:
        for bb in f.blocks:
            new, dirty = [], False
            for ins in bb.instructions:
                si = ins.sync_info
                if si is not None and len(si.on_wait) > limit:
                    waits = list(si.on_wait)
                    for w in waits[:-limit]:
                        _ctr[0] += 1
                        no = mybir.InstNoOp(name=f"zwaitnop-{_ctr[0]}", ins=[], outs=[])
                        no.engine = ins.engine
                        no.sync_info = mybir.SyncInfo(on_wait=[w], on_update=[])
                        new.append(no)
                    ins.sync_info = mybir.SyncInfo(
                        on_wait=waits[-limit:], on_update=list(si.on_update)
                    )
                    dirty = True
                new.append(ins)
            if dirty:
                bb.instructions = new


def _build():
    nc = bass.Bass(num_devices=N_CORES)
    xh_in = nc.declare_dram_parameter("xh", [SL, C, N], F16, isOutput=False)
    xl_in = nc.declare_dram_parameter("xl", [SL, C, N], F16, isOutput=False)
    xhb_in = nc.declare_dram_parameter("xhb", [SL, C, N], BF16, isOutput=False)
    wh_in = nc.declare_dram_parameter("whT", [C, C], F16, isOutput=False)
    wl_in = nc.declare_dram_parameter("wlT", [C, C], BF16, isOutput=False)
    g_in = nc.declare_dram_parameter("gamma", [C], F32, isOutput=False)
    b_in = nc.declare_dram_parameter("beta", [C], F32, isOutput=False)
    out = nc.declare_dram_parameter("out", [SL, N * C], U8, isOutput=True)

    xh_v = xh_in.rearrange("s (chh p) j -> s p chh j", chh=2, p=P)
    xl_v = xl_in.rearrange("s (chh p) j -> s p chh j", chh=2, p=P)
    xhb_v = xhb_in.rearrange("s (chh p) j -> s p chh j", chh=2, p=P)
    wh_v = wh_in.rearrange("(chh p) o -> p chh o", chh=2, p=P)
    wl_v = wl_in.rearrange("(chh p) o -> p chh o", chh=2, p=P)
    g_v = g_in.rearrange("(oh p) -> p oh", p=P)
    b_v = b_in.rearrange("(oh p) -> p oh", p=P)
    # out[sl] flat k = 65536*r + 256*(128*oh + p) + q holds spike(c, n=4q+r)
    # with SBUF free j = 256*r + q on partition p of half oh.
    out_v = out.rearrange("s (r ohh p q) -> s ohh p r q", r=4, ohh=2, p=P, q=256)

    with ExitStack() as ctx:
        tc = ctx.enter_context(tile.TileContext(nc))
        consts = ctx.enter_context(tc.tile_pool(name="consts", bufs=1))
        xhp = ctx.enter_context(tc.tile_pool(name="xhp", bufs=2))
        xlp = ctx.enter_context(tc.tile_pool(name="xlp", bufs=2))
        xhbp = ctx.enter_context(tc.tile_pool(name="xhbp", bufs=1))
        yps = ctx.enter_context(tc.tile_pool(name="yps", bufs=8, space="PSUM"))
        ysb = ctx.enter_context(tc.tile_pool(name="ysb", bufs=1))
        hp = ctx.enter_context(tc.tile_pool(name="hp", bufs=B_LOC))
        rdp = ctx.enter_context(tc.tile_pool(name="rdp", bufs=2))
        sp = ctx.enter_context(tc.tile_pool(name="sp", bufs=2))
        smallp = ctx.enter_context(tc.tile_pool(name="smallp", bufs=1))

        # ---- constants ----
        wh = consts.tile([P, 2, C], F16, name="wh")
        nc.sync.dma_start(out=wh, in_=wh_v)
        wl = consts.tile([P, 2, C], BF16, name="wl")
        nc.sync.dma_start(out=wl, in_=wl_v)
        gam = consts.tile([P, 2], F32)
        nc.sync.dma_start(out=gam, in_=g_v)
        bet = consts.tile([P, 2], F32)
        nc.sync.dma_start(out=bet, in_=b_v)

        y_sb = ysb.tile([P, 2, SL, N], F32)          # 128KB/partition
        stat6 = smallp.tile([P, 2, 2 * SL, 6], F32, name="stat6")

        # ---- phase 1: hi-term matmul, y1 -> SBUF, bn_stats ----
        for sl in range(SL):
            xh_t = xhp.tile([P, 2, N], F16, name="xh_t")
            nc.sync.dma_start(out=xh_t, in_=xh_v[sl])
            for oh in range(2):
                for nsl in range(2):
                    yp = yps.tile([P, 512], F32, name="yq")
                    for chh in range(2):
                        nc.tensor.matmul(
                            yp,
                            wh[:, chh, oh * P:(oh + 1) * P],
                            xh_t[:, chh, nsl * 512:(nsl + 1) * 512],
                            start=(chh == 0),
                            stop=(chh == 1),
                        )
                    nc.vector.bn_stats(stat6[:, oh, sl * 2 + nsl, :], yp)
                    nc.scalar.copy(y_sb[:, oh, sl, nsl * 512:(nsl + 1) * 512], yp)

        # ---- BN stats of y1: per-core (sum, sumsq) -> AllReduce ----
        mv = smallp.tile([P, 2, 2], F32)
        for oh in range(2):
            nc.vector.bn_aggr(mv[:, oh, :], stat6[:, oh, :, :])
        ccs = smallp.tile([P, 4], F32)                 # [sum0, sum1, ssq0, ssq1]
        msq = smallp.tile([P, 2], F32)
        for oh in range(2):
            nc.vector.tensor_scalar(
                ccs[:, oh:oh + 1], mv[:, oh, 0:1], NS_CORE, None, Alu.mult
            )
            nc.vector.tensor_tensor(
                msq[:, oh:oh + 1], mv[:, oh, 0:1], mv[:, oh, 0:1], Alu.mult
            )
            nc.vector.scalar_tensor_tensor(
                ccs[:, 2 + oh:3 + oh], mv[:, oh, 1:2], NS_CORE, msq[:, oh:oh + 1],
                Alu.bypass, Alu.add,
            )
        nc.vector.tensor_scalar(ccs[:, 2:4], ccs[:, 2:4], NS_CORE, None, Alu.mult)

        cc_in, _ = tc.tile([P, 4], F32, space="DRAM", name="cc_in")
        cc_out, _ = tc.tile([P, 4], F32, space="DRAM", addr_space="Shared", name="cc_out")
        nc.sync.dma_start(out=cc_in, in_=ccs)
        if not SINGLE:
            nc.gpsimd.collective_compute(
                "AllReduce", Alu.add,
                replica_groups=[list(range(N_CORES))],
                ins=[cc_in[:]], outs=[cc_out[:]],
            )
        gst = smallp.tile([P, 4], F32)
        nc.sync.dma_start(out=gst, in_=cc_in if SINGLE else cc_out)

        # ---- stats math (threshold units). yp1 = y1/2 stats: ----
        mean1 = smallp.tile([P, 2], F32)               # mean of yp1
        nc.vector.tensor_scalar(mean1, gst[:, 0:2], 1.0 / NS_TOT, None, Alu.mult)
        ey2 = smallp.tile([P, 2], F32)                 # E[yp1^2]
        nc.vector.tensor_scalar(ey2, gst[:, 2:4], 1.0 / NS_TOT, None, Alu.mult)
        nc.vector.tensor_tensor(msq, mean1, mean1, Alu.mult)
        u = smallp.tile([P, 2], F32)                   # var(y1_hat) + eps
        nc.vector.tensor_tensor(u, ey2, msq, Alu.subtract)
        nc.vector.tensor_scalar(u, u, 4.0, BN_EPS, Alu.mult, Alu.add)
        # rstd r = 1/sqrt(u), two Newton steps
        sq = smallp.tile([P, 2], F32)
        nc.scalar.sqrt(sq, u)
        r = smallp.tile([P, 2], F32)
        nc.vector.reciprocal(r, sq)
        t1 = smallp.tile([P, 2], F32)
        t2 = smallp.tile([P, 2], F32)
        for _ in range(2):
            nc.vector.tensor_tensor(t1, r, r, Alu.mult)
            nc.vector.tensor_tensor(t2, u, t1, Alu.mult)
            nc.vector.tensor_scalar(t2, t2, -0.5, 1.5, Alu.mult, Alu.add)
            nc.vector.tensor_tensor(r, r, t2, Alu.mult)
        # a = gamma * r ; theta = 1/a (Newton-refined) ; d = 0.5*beta*theta - mean1
        a_t = smallp.tile([P, 2], F32)
        nc.vector.tensor_tensor(a_t, gam, r, Alu.mult)
        theta = smallp.tile([P, 2], F32)
        nc.vector.reciprocal(theta, a_t)
        for _ in range(2):
            nc.vector.tensor_tensor(t1, a_t, theta, Alu.mult)
            nc.vector.tensor_scalar(t1, t1, -1.0, 2.0, Alu.mult, Alu.add)
            nc.vector.tensor_tensor(theta, theta, t1, Alu.mult)
        d_t = smallp.tile([P, 2], F32)
        nc.vector.tensor_tensor(t2, bet, theta, Alu.mult)
        nc.vector.scalar_tensor_tensor(d_t, t2, 0.5, mean1, Alu.mult, Alu.subtract)

        # ---- phase 2: residual matmuls + LIF, t-outer for bl pipelining ----
        hs = [hp.tile([P, 2, N], F32, name="h") for bl in range(B_LOC)]
        for t in range(T):
            for bl in range(B_LOC):
                sl = bl * 4 + t
                h = hs[bl]
                xl_t = xlp.tile([P, 2, N], F16, name="xl_t")
                nc.sync.dma_start(out=xl_t, in_=xl_v[sl])
                xhb_t = xhbp.tile([P, 2, N], BF16, name="xhb_t")
                nc.sync.dma_start(out=xhb_t, in_=xhb_v[sl])
                resid = {}
                for oh in range(2):
                    for nsl in range(2):
                        rp = yps.tile([P, 512], F32, name="yq")
                        i = 0
                        for w_t, x_t in ((wh, xl_t), (wl, xhb_t)):
                            for chh in range(2):
                                nc.tensor.matmul(
                                    rp,
                                    w_t[:, chh, oh * P:(oh + 1) * P],
                                    x_t[:, chh, nsl * 512:(nsl + 1) * 512],
                                    start=(i == 0),
                                    stop=(i == 3),
                                )
                                i += 1
                        resid[(oh, nsl)] = rp
                # resid_d = resid + d (ACT, PSUM -> SBUF; frees PSUM banks)
                rd = rdp.tile([P, 2, N], F32, name="rd")
                for oh in range(2):
                    for nsl in range(2):
                        nc.scalar.activation(
                            rd[:, oh, nsl * 512:(nsl + 1) * 512],
                            resid[(oh, nsl)], ActF.Identity,
                            bias=d_t[:, oh:oh + 1], scale=1.0,
                        )
                if t > 0:
                    # P1: h = 0.5*h + y1   (DVE-only: stt)
                    nc.vector.scalar_tensor_tensor(
                        h, h, 0.5, y_sb[:, :, sl, :], Alu.mult, Alu.add
                    )
                # P2: h = h + resid_d   (t=0: h = y1 + resid_d); plain tt
                p2_eng = nc.gpsimd if P2_ENG[sl] == "g" else nc.vector
                in0 = h if t > 0 else y_sb[:, :, sl, :]
                p2_eng.tensor_tensor(h, in0, rd, Alu.add)
                s = sp.tile([P, 2, N], U8, name="s")
                s_eng = nc.gpsimd if S_ENG[sl] == "g" else nc.vector
                for oh in range(2):
                    s_eng.tensor_scalar(
                        s[:, oh, :], h[:, oh, :], theta[:, oh:oh + 1], None, Alu.is_ge
                    )
                for oh in range(2):
                    nc.sync.dma_start(out=out_v[sl, oh], in_=s[:, oh, :])
                if t < 3:
                    # reset: h = (h < theta) * h   (DVE-only: stt)
                    for oh in range(2):
                        nc.vector.scalar_tensor_tensor(
                            h[:, oh, :], h[:, oh, :], theta[:, oh:oh + 1],
                            h[:, oh, :], Alu.is_lt, Alu.mult,
                        )

    if LEGALIZE:
        _legalize_waits(nc)
    return nc


_nc_cache = None


def _get_nc():
    global _nc_cache
    if _nc_cache is None:
        _nc_cache = _build()
    return _nc_cache


def _tb_index(core, sl):
    bl, t = sl // T, sl % T
    return t * B + core * B_LOC + bl


def _prep_core_inputs(x, core):
    """Per-core input prep: gather slices, transpose to c-major with the
    n -> (q, r) -> j = 256r + q permutation, split fp16 hi/lo + bf16 hi."""
    import ml_dtypes

    idx = [_tb_index(core, sl) for sl in range(SL)]
    xc = x[idx]                                  # (SL, N, C) f32
    xct = xc.transpose(0, 2, 1)                  # (SL, C, N)
    xct = xct.reshape(SL, C, 256, 4).transpose(0, 1, 3, 2).reshape(SL, C, N)
    xh = xct.astype(np.float16)
    xl = (xct - xh.astype(np.float32)).astype(np.float16)
    xhb = xh.astype(ml_dtypes.bfloat16)
    return (np.ascontiguousarray(xh), np.ascontiguousarray(xl),
            np.ascontiguousarray(xhb))


def kernel(x, W, gamma, beta, _trace=False, _trace_kwargs=None):
    import ml_dtypes
    from concourse.bass_utils import run_bass_kernel_spmd

    x = np.ascontiguousarray(np.asarray(x, dtype=np.float32))
    W = np.ascontiguousarray(np.asarray(W, dtype=np.float32))
    gamma = np.ascontiguousarray(np.asarray(gamma, dtype=np.float32))
    beta = np.ascontiguousarray(np.asarray(beta, dtype=np.float32))

    Wh = W.astype(np.float16)
    Wl = (W - Wh.astype(np.float32)).astype(np.float16)
    whT = np.ascontiguousarray((Wh * np.float16(0.5)).T)
    wlT = np.ascontiguousarray(
        (Wl.astype(np.float32) * 0.5).astype(ml_dtypes.bfloat16).T
    )

    nc = _get_nc()
    in_maps = []
    for k in range(N_CORES):
        xh, xl, xhb = _prep_core_inputs(x, k)
        in_maps.append({
            "xh": xh, "xl": xl, "xhb": xhb,
            "whT": whT, "wlT": wlT, "gamma": gamma, "beta": beta,
        })
    kwargs = dict(_trace_kwargs or {})
    res = run_bass_kernel_spmd(
        nc, in_maps, core_ids=list(range(N_CORES)), trace=_trace, **kwargs
    )
    out = np.empty((T * B, N, C), dtype=np.float32)
    for k in range(N_CORES):
        ok = res.results[k]["out"]
        for sl in range(SL):
            out[_tb_index(k, sl)] = ok[sl].reshape(N, C).astype(np.float32)
    if _trace:
        return out, res
    return out


# revision 13
# speedup vs baseline: 1.5810x; 1.5810x over previous
"""Trainium2 Bass kernel for nn_Decoder1 (linear -> BatchNorm1d -> multistep LIF).

Reference computation (T=4, B=32, N=1024, C=256):
  y[tb,o,n]   = sum_c x[tb,n,c] * W[o,c]                      (TB=128 slices)
  z           = BN(y) over (tb, n) per channel o (training stats, eps=1e-5)
  LIF over t  : v' = (v + z_t)/2 ; s = (v' >= 1) ; v' *= (1-s)
  out[tb,n',c'] = spikes[tb].reshape(C,N).T   (row-major reinterpretation)

Design (v3):
  - Data-parallel over B: 4 b-values x 4 timesteps = 16 (N,C) slices/core.
  - Matmul via fp16 3-term split: y = xh@Wh + xl@Wh + xhb@Wl (xh=fp16(x),
    xl=fp16(x-xh), xhb=bf16(xh); split + c-major transpose done on HOST,
    so no PE transposes; W pre-scaled by 0.5 so PSUM = yp = y/2).
  - Phase 1: hi-term only (y1 = xh@Wh'), bn_stats from PSUM, y1 DISCARDED.
    Stats bias vs full y is ~95 spike flips, rel err ~0.0095 (measured).
  - Tiny AllReduce of (sum, sumsq) -> a = gamma*rstd, bias = 0.5*beta -
    mean1*a. LIF runs in v-units so spike/reset compare against the
    IMMEDIATE 1.0 (AP-scalar compare ops measured 5-10x slower than
    immediate ones on DVE/GpSimd -- avoid them in hot loops).
  - Phase 2 (mms don't wait for the AR; zd does): recompute all 3 terms
    into one PSUM quadrant, zd = ACT Identity(scale=a, bias)(yp) (ACT
    applies per-channel affines fast), v = 0.5v + zd (stt, immediate),
    s = (v >= 1.0) -> u8 (immediate), reset v = (v < 1.0)*v (immediate).
    Spikes leave as uint8 (4x less DMA); host expands to f32.
"""

import numpy as np
from contextlib import ExitStack

import concourse.bass as bass
import concourse.mybir as mybir
import concourse.tile as tile

F32 = mybir.dt.float32
F16 = mybir.dt.float16
BF16 = mybir.dt.bfloat16
U8 = mybir.dt.uint8
Alu = mybir.AluOpType
ActF = mybir.ActivationFunctionType

N_CORES = 8
T, B, N, C = 4, 32, 1024, 256
B_LOC = B // N_CORES            # 4 batch entries per core
SL = T * B_LOC                  # 16 (N,C) slices per core; sl = bl*4 + t
P = 128
NS_CORE = float(SL * N)         # BN samples per channel per core
NS_TOT = float(T * B * N)       # BN samples per channel globally
BN_EPS = 1e-5

_ctr = [0]
SINGLE = False   # test-only: skip the AllReduce (single-core sim)
LEGALIZE = True  # set False for CoreSim (NoOp waits trip the race detector)
# engine per slice for the spike compare (v=vector, g=gpsimd) -- immediate
# scalar both ways; half/half to measure gpsimd's imm tensor_scalar rate.
S_ENG = {sl: ("g" if sl % 2 == 0 else "v") for sl in range(SL)}


def _legalize_waits(nc, limit=1):
    """Hoist excess semaphore waits onto same-engine NoOps (walrus accepts
    very few waits per instruction; PE matmul: 1)."""
    for f in nc.m.functions:
        for bb in f.blocks:
            new, dirty = [], False
            for ins in bb.instructions:
                si = ins.sync_info
                if si is not None and len(si.on_wait) > limit:
                    waits = list(si.on_wait)
                    for w in waits[:-limit]:
                        _ctr[0] += 1
                        no = mybir.InstNoOp(name=f"zwaitnop-{_ctr[0]}", ins=[], outs=[])
                        no.engine = ins.engine
                        no.sync_info = mybir.SyncInfo(on_wait=[w], on_update=[])
                        new.append(no)
                    ins.sync_info = mybir.SyncInfo(
                        on_wait=waits[-limit:], on_update=list(si.on_update)
                    )
                    dirty = True
                new.append(ins)
            if dirty:
                bb.instructions = new


def _build():
    nc = bass.Bass(num_devices=N_CORES)
    xh_in = nc.declare_dram_parameter("xh", [SL, C, N], F16, isOutput=False)
    xl_in = nc.declare_dram_parameter("xl", [SL, C, N], F16, isOutput=False)
    xhb_in = nc.declare_dram_parameter("xhb", [SL, C, N], BF16, isOutput=False)
    wh_in = nc.declare_dram_parameter("whT", [C, C], F16, isOutput=False)
    wl_in = nc.declare_dram_parameter("wlT", [C, C], BF16, isOutput=False)
    g_in = nc.declare_dram_parameter("gamma", [C], F32, isOutput=False)
    b_in = nc.declare_dram_parameter("beta", [C], F32, isOutput=False)
    out = nc.declare_dram_parameter("out", [SL, N * C], U8, isOutput=True)

    xh_v = xh_in.rearrange("s (chh p) j -> s p chh j", chh=2, p=P)
    xl_v = xl_in.rearrange("s (chh p) j -> s p chh j", chh=2, p=P)
    xhb_v = xhb_in.rearrange("s (chh p) j -> s p chh j", chh=2, p=P)
    wh_v = wh_in.rearrange("(chh p) o -> p chh o", chh=2, p=P)
    wl_v = wl_in.rearrange("(chh p) o -> p chh o", chh=2, p=P)
    g_v = g_in.rearrange("(oh p) -> p oh", p=P)
    b_v = b_in.rearrange("(oh p) -> p oh", p=P)
    # out[sl] flat k = 65536*r + 256*(128*oh + p) + q holds spike(c, n=4q+r)
    # with SBUF free j = 256*r + q on partition p of half oh.
    out_v = out.rearrange("s (r ohh p q) -> s ohh p r q", r=4, ohh=2, p=P, q=256)

    with ExitStack() as ctx:
        tc = ctx.enter_context(tile.TileContext(nc))
        consts = ctx.enter_context(tc.tile_pool(name="consts", bufs=1))
        xhp = ctx.enter_context(tc.tile_pool(name="xhp", bufs=3))
        xlp = ctx.enter_context(tc.tile_pool(name="xlp", bufs=3))
        xhbp = ctx.enter_context(tc.tile_pool(name="xhbp", bufs=3))
        yps = ctx.enter_context(tc.tile_pool(name="yps", bufs=8, space="PSUM"))
        vp = ctx.enter_context(tc.tile_pool(name="vp", bufs=B_LOC))
        zdp = ctx.enter_context(tc.tile_pool(name="zdp", bufs=3))
        sp = ctx.enter_context(tc.tile_pool(name="sp", bufs=3))
        smallp = ctx.enter_context(tc.tile_pool(name="smallp", bufs=1))

        # ---- constants ----
        wh = consts.tile([P, 2, C], F16, name="wh")
        nc.sync.dma_start(out=wh, in_=wh_v)
        wl = consts.tile([P, 2, C], BF16, name="wl")
        nc.sync.dma_start(out=wl, in_=wl_v)
        gam = consts.tile([P, 2], F32)
        nc.sync.dma_start(out=gam, in_=g_v)
        bet = consts.tile([P, 2], F32)
        nc.sync.dma_start(out=bet, in_=b_v)

        stat6 = smallp.tile([P, 2, 2 * SL, 6], F32, name="stat6")

        # ---- phase 1: hi-term matmul for BN stats only (y1 discarded) ----
        for sl in range(SL):
            xh_t = xhp.tile([P, 2, N], F16, name="xh_t")
            nc.sync.dma_start(out=xh_t, in_=xh_v[sl])
            for oh in range(2):
                for nsl in range(2):
                    yp = yps.tile([P, 512], F32, name="yq")
                    for chh in range(2):
                        nc.tensor.matmul(
                            yp,
                            wh[:, chh, oh * P:(oh + 1) * P],
                            xh_t[:, chh, nsl * 512:(nsl + 1) * 512],
                            start=(chh == 0),
                            stop=(chh == 1),
                        )
                    nc.vector.bn_stats(stat6[:, oh, sl * 2 + nsl, :], yp)

        # ---- BN stats of y1: per-core (sum, sumsq) -> AllReduce ----
        mv = smallp.tile([P, 2, 2], F32)
        for oh in range(2):
            nc.vector.bn_aggr(mv[:, oh, :], stat6[:, oh, :, :])
        ccs = smallp.tile([P, 4], F32)                 # [sum0, sum1, ssq0, ssq1]
        msq = smallp.tile([P, 2], F32)
        for oh in range(2):
            nc.vector.tensor_scalar(
                ccs[:, oh:oh + 1], mv[:, oh, 0:1], NS_CORE, None, Alu.mult
            )
            nc.vector.tensor_tensor(
                msq[:, oh:oh + 1], mv[:, oh, 0:1], mv[:, oh, 0:1], Alu.mult
            )
            nc.vector.scalar_tensor_tensor(
                ccs[:, 2 + oh:3 + oh], mv[:, oh, 1:2], NS_CORE, msq[:, oh:oh + 1],
                Alu.bypass, Alu.add,
            )
        nc.vector.tensor_scalar(ccs[:, 2:4], ccs[:, 2:4], NS_CORE, None, Alu.mult)

        cc_in, _ = tc.tile([P, 4], F32, space="DRAM", name="cc_in")
        cc_out, _ = tc.tile([P, 4], F32, space="DRAM", addr_space="Shared", name="cc_out")
        nc.sync.dma_start(out=cc_in, in_=ccs)
        if not SINGLE:
            nc.gpsimd.collective_compute(
                "AllReduce", Alu.add,
                replica_groups=[list(range(N_CORES))],
                ins=[cc_in[:]], outs=[cc_out[:]],
            )
        gst = smallp.tile([P, 4], F32)
        nc.sync.dma_start(out=gst, in_=cc_in if SINGLE else cc_out)

        # ---- stats math. yp1 = y1_hat/2 stats: ----
        mean1 = smallp.tile([P, 2], F32)               # mean of yp1
        nc.vector.tensor_scalar(mean1, gst[:, 0:2], 1.0 / NS_TOT, None, Alu.mult)
        ey2 = smallp.tile([P, 2], F32)                 # E[yp1^2]
        nc.vector.tensor_scalar(ey2, gst[:, 2:4], 1.0 / NS_TOT, None, Alu.mult)
        nc.vector.tensor_tensor(msq, mean1, mean1, Alu.mult)
        u = smallp.tile([P, 2], F32)                   # var(y1_hat) + eps
        nc.vector.tensor_tensor(u, ey2, msq, Alu.subtract)
        nc.vector.tensor_scalar(u, u, 4.0, BN_EPS, Alu.mult, Alu.add)
        # rstd r = 1/sqrt(u), two Newton steps
        sq = smallp.tile([P, 2], F32)
        nc.scalar.sqrt(sq, u)
        r = smallp.tile([P, 2], F32)
        nc.vector.reciprocal(r, sq)
        t1 = smallp.tile([P, 2], F32)
        t2 = smallp.tile([P, 2], F32)
        for _ in range(2):
            nc.vector.tensor_tensor(t1, r, r, Alu.mult)
            nc.vector.tensor_tensor(t2, u, t1, Alu.mult)
            nc.vector.tensor_scalar(t2, t2, -0.5, 1.5, Alu.mult, Alu.add)
            nc.vector.tensor_tensor(r, r, t2, Alu.mult)
        # a = gamma * r ; zd bias = 0.5*beta - mean1*a
        a_t = smallp.tile([P, 2], F32)
        nc.vector.tensor_tensor(a_t, gam, r, Alu.mult)
        zb = smallp.tile([P, 2], F32)
        nc.vector.tensor_tensor(t1, mean1, a_t, Alu.mult)
        nc.vector.scalar_tensor_tensor(zb, bet, 0.5, t1, Alu.mult, Alu.subtract)

        # ---- phase 2: full 3-term matmuls + LIF, t-outer for bl pipeline ----
        vs = [vp.tile([P, 2, N], F32, name="v") for bl in range(B_LOC)]
        for t in range(T):
            for bl in range(B_LOC):
                sl = bl * 4 + t
                v = vs[bl]
                xh2_t = xhp.tile([P, 2, N], F16, name="xh_t")
                nc.sync.dma_start(out=xh2_t, in_=xh_v[sl])
                xl_t = xlp.tile([P, 2, N], F16, name="xl_t")
                nc.sync.dma_start(out=xl_t, in_=xl_v[sl])
                xhb_t = xhbp.tile([P, 2, N], BF16, name="xhb_t")
                nc.sync.dma_start(out=xhb_t, in_=xhb_v[sl])
                # zd written into v directly at t=0 (v = zd when v_prev = 0)
                zd = v if t == 0 else zdp.tile([P, 2, N], F32, name="zd")
                for oh in range(2):
                    for nsl in range(2):
                        yp = yps.tile([P, 512], F32, name="yq")
                        i = 0
                        for w_t, chh, x_t in (
                            (wh, 0, xh2_t), (wh, 0, xl_t),
                            (wh, 1, xh2_t), (wh, 1, xl_t),
                            (wl, 0, xhb_t), (wl, 1, xhb_t),
                        ):
                            nc.tensor.matmul(
                                yp,
                                w_t[:, chh, oh * P:(oh + 1) * P],
                                x_t[:, chh, nsl * 512:(nsl + 1) * 512],
                                start=(i == 0),
                                stop=(i == 5),
                            )
                            i += 1
                        # zd = a*yp + (0.5*beta - mean1*a)   [ACT per-channel]
                        nc.scalar.activation(
                            zd[:, oh, nsl * 512:(nsl + 1) * 512], yp,
                            ActF.Identity,
                            bias=zb[:, oh:oh + 1], scale=a_t[:, oh:oh + 1],
                        )
                if t > 0:
                    # v = 0.5*v + zd  (immediate-scalar stt)
                    nc.vector.scalar_tensor_tensor(
                        v, v, 0.5, zd, Alu.mult, Alu.add
                    )
                s = sp.tile([P, 2, N], U8, name="s")
                s_eng = nc.gpsimd if S_ENG[sl] == "g" else nc.vector
                s_eng.tensor_scalar(s, v, 1.0, None, Alu.is_ge)
                for oh in range(2):
                    nc.sync.dma_start(out=out_v[sl, oh], in_=s[:, oh, :])
                if t < 3:
                    # reset: v = (v < 1.0) * v  (immediate-scalar stt)
                    nc.vector.scalar_tensor_tensor(
                        v, v, 1.0, v, Alu.is_lt, Alu.mult
                    )

    if LEGALIZE:
        _legalize_waits(nc)
    return nc


_nc_cache = None


def _get_nc():
    global _nc_cache
    if _nc_cache is None:
        _nc_cache = _build()
    return _nc_cache


def _tb_index(core, sl):
    bl, t = sl // T, sl % T
    return t * B + core * B_LOC + bl


def _prep_core_inputs(x, core):
    """Per-core input prep: gather slices, transpose to c-major with the
    n -> (q, r) -> j = 256r + q permutation, split fp16 hi/lo + bf16 hi."""
    import ml_dtypes

    idx = [_tb_index(core, sl) for sl in range(SL)]
    xc = x[idx]                                  # (SL, N, C) f32
    xct = xc.transpose(0, 2, 1)                  # (SL, C, N)
    xct = xct.reshape(SL, C, 256, 4).transpose(0, 1, 3, 2).reshape(SL, C, N)
    xh = xct.astype(np.float16)
    xl = (xct - xh.astype(np.float32)).astype(np.float16)
    xhb = xh.astype(ml_dtypes.bfloat16)
    return (np.ascontiguousarray(xh), np.ascontiguousarray(xl),
            np.ascontiguousarray(xhb))


def kernel(x, W, gamma, beta, _trace=False, _trace_kwargs=None):
    import ml_dtypes
    from concourse.bass_utils import run_bass_kernel_spmd

    x = np.ascontiguousarray(np.asarray(x, dtype=np.float32))
    W = np.ascontiguousarray(np.asarray(W, dtype=np.float32))
    gamma = np.ascontiguousarray(np.asarray(gamma, dtype=np.float32))
    beta = np.ascontiguousarray(np.asarray(beta, dtype=np.float32))

    Wh = W.astype(np.float16)
    Wl = (W - Wh.astype(np.float32)).astype(np.float16)
    whT = np.ascontiguousarray((Wh * np.float16(0.5)).T)
    wlT = np.ascontiguousarray(
        (Wl.astype(np.float32) * 0.5).astype(ml_dtypes.bfloat16).T
    )

    nc = _get_nc()
    in_maps = []
    for k in range(N_CORES):
        xh, xl, xhb = _prep_core_inputs(x, k)
        in_maps.append({
            "xh": xh, "xl": xl, "xhb": xhb,
            "whT": whT, "wlT": wlT, "gamma": gamma, "beta": beta,
        })
    kwargs = dict(_trace_kwargs or {})
    res = run_bass_kernel_spmd(
        nc, in_maps, core_ids=list(range(N_CORES)), trace=_trace, **kwargs
    )
    out = np.empty((T * B, N, C), dtype=np.float32)
    for k in range(N_CORES):
        ok = res.results[k]["out"]
        for sl in range(SL):
            out[_tb_index(k, sl)] = ok[sl].reshape(N, C).astype(np.float32)
    if _trace:
        return out, res
    return out


# revision 18
# speedup vs baseline: 3.2923x; 2.0824x over previous
"""Trainium2 Bass kernel for nn_Decoder1 (linear -> BatchNorm1d -> multistep LIF).

Reference computation (T=4, B=32, N=1024, C=256):
  y[tb,o,n]   = sum_c x[tb,n,c] * W[o,c]                      (TB=128 slices)
  z           = BN(y) over (tb, n) per channel o (training stats, eps=1e-5)
  LIF over t  : v' = (v + z_t)/2 ; s = (v' >= 1) ; v' *= (1-s)
  out[tb,n',c'] = spikes[tb].reshape(C,N).T   (row-major reinterpretation)

Design (v3):
  - Data-parallel over B: 4 b-values x 4 timesteps = 16 (N,C) slices/core.
  - Matmul via fp16 3-term split: y = xh@Wh + xl@Wh + xhb@Wl (xh=fp16(x),
    xl=fp16(x-xh), xhb=bf16(xh); split + c-major transpose done on HOST,
    so no PE transposes; W pre-scaled by 0.5 so PSUM = yp = y/2).
  - Phase 1: hi-term only (y1 = xh@Wh'), bn_stats from PSUM, y1 DISCARDED.
    Stats bias vs full y is ~95 spike flips, rel err ~0.0095 (measured).
  - Tiny AllReduce of (sum, sumsq) -> a = gamma*rstd, bias = 0.5*beta -
    mean1*a. LIF runs in v-units so spike/reset compare against the
    IMMEDIATE 1.0 (AP-scalar compare ops measured 5-10x slower than
    immediate ones on DVE/GpSimd -- avoid them in hot loops).
  - Phase 2 (mms don't wait for the AR; zd does): recompute all 3 terms
    into one PSUM quadrant, zd = ACT Identity(scale=a, bias)(yp) (ACT
    applies per-channel affines fast), v = 0.5v + zd (stt, immediate),
    s = (v >= 1.0) -> u8 (immediate), reset v = (v < 1.0)*v (immediate).
    Spikes leave as uint8 (4x less DMA); host expands to f32.
"""

import numpy as np
from contextlib import ExitStack

import concourse.bass as bass
import concourse.mybir as mybir
import concourse.tile as tile

F32 = mybir.dt.float32
F16 = mybir.dt.float16
BF16 = mybir.dt.bfloat16
U8 = mybir.dt.uint8
Alu = mybir.AluOpType
ActF = mybir.ActivationFunctionType

N_CORES = 8
T, B, N, C = 4, 32, 1024, 256
B_LOC = B // N_CORES            # 4 batch entries per core
SL = T * B_LOC                  # 16 (N,C) slices per core; sl = bl*4 + t
P = 128
NS_CORE = float(SL * N)         # BN samples per channel per core
NS_TOT = float(T * B * N)       # BN samples per channel globally
BN_EPS = 1e-5

_ctr = [0]
SINGLE = False   # test-only: skip the AllReduce (single-core sim)
LEGALIZE = True  # set False for CoreSim (NoOp waits trip the race detector)
# GpSimd elementwise is hopeless (31 us per [128,2048] op measured) -- all
# LIF elementwise runs on DVE. The in-place 3-way-aliased reset stt
# (v,1.0,v)->v measured 9.6 us vs ~2 for other stt; test two variants:
# "smask": v = (s_u8 == 0) * v (distinct in0)  |  "fresh": out to new tile
RESET_VARIANT = {bl: ("smask" if bl < 2 else "fresh") for bl in range(B_LOC)}


def _legalize_waits(nc, limit=1):
    """Hoist excess semaphore waits onto same-engine NoOps (walrus accepts
    very few waits per instruction; PE matmul: 1)."""
    for f in nc.m.functions:
        for bb in f.blocks:
            new, dirty = [], False
            for ins in bb.instructions:
                si = ins.sync_info
                if si is not None and len(si.on_wait) > limit:
                    waits = list(si.on_wait)
                    for w in waits[:-limit]:
                        _ctr[0] += 1
                        no = mybir.InstNoOp(name=f"zwaitnop-{_ctr[0]}", ins=[], outs=[])
                        no.engine = ins.engine
                        no.sync_info = mybir.SyncInfo(on_wait=[w], on_update=[])
                        new.append(no)
                    ins.sync_info = mybir.SyncInfo(
                        on_wait=waits[-limit:], on_update=list(si.on_update)
                    )
                    dirty = True
                new.append(ins)
            if dirty:
                bb.instructions = new


def _build():
    nc = bass.Bass(num_devices=N_CORES)
    xh_in = nc.declare_dram_parameter("xh", [SL, C, N], F16, isOutput=False)
    xl_in = nc.declare_dram_parameter("xl", [SL, C, N], F16, isOutput=False)
    xhb_in = nc.declare_dram_parameter("xhb", [SL, C, N], BF16, isOutput=False)
    wh_in = nc.declare_dram_parameter("whT", [C, C], F16, isOutput=False)
    wl_in = nc.declare_dram_parameter("wlT", [C, C], BF16, isOutput=False)
    g_in = nc.declare_dram_parameter("gamma", [C], F32, isOutput=False)
    b_in = nc.declare_dram_parameter("beta", [C], F32, isOutput=False)
    out = nc.declare_dram_parameter("out", [SL, N * C], U8, isOutput=True)

    xh_v = xh_in.rearrange("s (chh p) j -> s p chh j", chh=2, p=P)
    xl_v = xl_in.rearrange("s (chh p) j -> s p chh j", chh=2, p=P)
    xhb_v = xhb_in.rearrange("s (chh p) j -> s p chh j", chh=2, p=P)
    wh_v = wh_in.rearrange("(chh p) o -> p chh o", chh=2, p=P)
    wl_v = wl_in.rearrange("(chh p) o -> p chh o", chh=2, p=P)
    g_v = g_in.rearrange("(oh p) -> p oh", p=P)
    b_v = b_in.rearrange("(oh p) -> p oh", p=P)
    # out[sl] flat k = 65536*r + 256*(128*oh + p) + q holds spike(c, n=4q+r)
    # with SBUF free j = 256*r + q on partition p of half oh.
    out_v = out.rearrange("s (r ohh p q) -> s ohh p r q", r=4, ohh=2, p=P, q=256)

    with ExitStack() as ctx:
        tc = ctx.enter_context(tile.TileContext(nc))
        consts = ctx.enter_context(tc.tile_pool(name="consts", bufs=1))
        xhp = ctx.enter_context(tc.tile_pool(name="xhp", bufs=3))
        xlp = ctx.enter_context(tc.tile_pool(name="xlp", bufs=3))
        xhbp = ctx.enter_context(tc.tile_pool(name="xhbp", bufs=3))
        yps = ctx.enter_context(tc.tile_pool(name="yps", bufs=8, space="PSUM"))
        vp = ctx.enter_context(tc.tile_pool(name="vp", bufs=8))
        zdp = ctx.enter_context(tc.tile_pool(name="zdp", bufs=3))
        sp = ctx.enter_context(tc.tile_pool(name="sp", bufs=3))
        smallp = ctx.enter_context(tc.tile_pool(name="smallp", bufs=1))

        # ---- constants ----
        wh = consts.tile([P, 2, C], F16, name="wh")
        nc.sync.dma_start(out=wh, in_=wh_v)
        wl = consts.tile([P, 2, C], BF16, name="wl")
        nc.sync.dma_start(out=wl, in_=wl_v)
        gam = consts.tile([P, 2], F32)
        nc.sync.dma_start(out=gam, in_=g_v)
        bet = consts.tile([P, 2], F32)
        nc.sync.dma_start(out=bet, in_=b_v)

        stat6 = smallp.tile([P, 2, 2 * SL, 6], F32, name="stat6")

        # ---- phase 1: hi-term matmul for BN stats only (y1 discarded) ----
        for sl in range(SL):
            xh_t = xhp.tile([P, 2, N], F16, name="xh_t")
            nc.sync.dma_start(out=xh_t, in_=xh_v[sl])
            for oh in range(2):
                for nsl in range(2):
                    yp = yps.tile([P, 512], F32, name="yq")
                    for chh in range(2):
                        nc.tensor.matmul(
                            yp,
                            wh[:, chh, oh * P:(oh + 1) * P],
                            xh_t[:, chh, nsl * 512:(nsl + 1) * 512],
                            start=(chh == 0),
                            stop=(chh == 1),
                        )
                    nc.vector.bn_stats(stat6[:, oh, sl * 2 + nsl, :], yp)

        # ---- BN stats of y1: per-core (sum, sumsq) -> AllReduce ----
        mv = smallp.tile([P, 2, 2], F32)
        for oh in range(2):
            nc.vector.bn_aggr(mv[:, oh, :], stat6[:, oh, :, :])
        ccs = smallp.tile([P, 4], F32)                 # [sum0, sum1, ssq0, ssq1]
        msq = smallp.tile([P, 2], F32)
        for oh in range(2):
            nc.vector.tensor_scalar(
                ccs[:, oh:oh + 1], mv[:, oh, 0:1], NS_CORE, None, Alu.mult
            )
            nc.vector.tensor_tensor(
                msq[:, oh:oh + 1], mv[:, oh, 0:1], mv[:, oh, 0:1], Alu.mult
            )
            nc.vector.scalar_tensor_tensor(
                ccs[:, 2 + oh:3 + oh], mv[:, oh, 1:2], NS_CORE, msq[:, oh:oh + 1],
                Alu.bypass, Alu.add,
            )
        nc.vector.tensor_scalar(ccs[:, 2:4], ccs[:, 2:4], NS_CORE, None, Alu.mult)

        cc_in, _ = tc.tile([P, 4], F32, space="DRAM", name="cc_in")
        cc_out, _ = tc.tile([P, 4], F32, space="DRAM", addr_space="Shared", name="cc_out")
        nc.sync.dma_start(out=cc_in, in_=ccs)
        if not SINGLE:
            nc.gpsimd.collective_compute(
                "AllReduce", Alu.add,
                replica_groups=[list(range(N_CORES))],
                ins=[cc_in[:]], outs=[cc_out[:]],
            )
        gst = smallp.tile([P, 4], F32)
        nc.sync.dma_start(out=gst, in_=cc_in if SINGLE else cc_out)

        # ---- stats math. yp1 = y1_hat/2 stats: ----
        mean1 = smallp.tile([P, 2], F32)               # mean of yp1
        nc.vector.tensor_scalar(mean1, gst[:, 0:2], 1.0 / NS_TOT, None, Alu.mult)
        ey2 = smallp.tile([P, 2], F32)                 # E[yp1^2]
        nc.vector.tensor_scalar(ey2, gst[:, 2:4], 1.0 / NS_TOT, None, Alu.mult)
        nc.vector.tensor_tensor(msq, mean1, mean1, Alu.mult)
        u = smallp.tile([P, 2], F32)                   # var(y1_hat) + eps
        nc.vector.tensor_tensor(u, ey2, msq, Alu.subtract)
        nc.vector.tensor_scalar(u, u, 4.0, BN_EPS, Alu.mult, Alu.add)
        # rstd r = 1/sqrt(u), two Newton steps
        sq = smallp.tile([P, 2], F32)
        nc.scalar.sqrt(sq, u)
        r = smallp.tile([P, 2], F32)
        nc.vector.reciprocal(r, sq)
        t1 = smallp.tile([P, 2], F32)
        t2 = smallp.tile([P, 2], F32)
        for _ in range(2):
            nc.vector.tensor_tensor(t1, r, r, Alu.mult)
            nc.vector.tensor_tensor(t2, u, t1, Alu.mult)
            nc.vector.tensor_scalar(t2, t2, -0.5, 1.5, Alu.mult, Alu.add)
            nc.vector.tensor_tensor(r, r, t2, Alu.mult)
        # a = gamma * r ; zd bias = 0.5*beta - mean1*a
        a_t = smallp.tile([P, 2], F32)
        nc.vector.tensor_tensor(a_t, gam, r, Alu.mult)
        zb = smallp.tile([P, 2], F32)
        nc.vector.tensor_tensor(t1, mean1, a_t, Alu.mult)
        nc.vector.scalar_tensor_tensor(zb, bet, 0.5, t1, Alu.mult, Alu.subtract)

        # ---- phase 2: full 3-term matmuls + LIF, t-outer for bl pipeline ----
        vs = [None] * B_LOC
        for t in range(T):
            for bl in range(B_LOC):
                sl = bl * 4 + t
                if t == 0:
                    vs[bl] = vp.tile([P, 2, N], F32, name="v")
                v = vs[bl]
                xh2_t = xhp.tile([P, 2, N], F16, name="xh_t")
                nc.sync.dma_start(out=xh2_t, in_=xh_v[sl])
                xl_t = xlp.tile([P, 2, N], F16, name="xl_t")
                nc.sync.dma_start(out=xl_t, in_=xl_v[sl])
                xhb_t = xhbp.tile([P, 2, N], BF16, name="xhb_t")
                nc.sync.dma_start(out=xhb_t, in_=xhb_v[sl])
                # zd written into v directly at t=0 (v = zd when v_prev = 0)
                zd = v if t == 0 else zdp.tile([P, 2, N], F32, name="zd")
                for oh in range(2):
                    for nsl in range(2):
                        yp = yps.tile([P, 512], F32, name="yq")
                        i = 0
                        for w_t, chh, x_t in (
                            (wh, 0, xh2_t), (wh, 0, xl_t),
                            (wh, 1, xh2_t), (wh, 1, xl_t),
                            (wl, 0, xhb_t), (wl, 1, xhb_t),
                        ):
                            nc.tensor.matmul(
                                yp,
                                w_t[:, chh, oh * P:(oh + 1) * P],
                                x_t[:, chh, nsl * 512:(nsl + 1) * 512],
                                start=(i == 0),
                                stop=(i == 5),
                            )
                            i += 1
                        # zd = a*yp + (0.5*beta - mean1*a)   [ACT per-channel]
                        nc.scalar.activation(
                            zd[:, oh, nsl * 512:(nsl + 1) * 512], yp,
                            ActF.Identity,
                            bias=zb[:, oh:oh + 1], scale=a_t[:, oh:oh + 1],
                        )
                if t > 0:
                    # v = 0.5*v + zd  (immediate-scalar stt)
                    nc.vector.scalar_tensor_tensor(
                        v, v, 0.5, zd, Alu.mult, Alu.add
                    )
                s = sp.tile([P, 2, N], U8, name="s")
                nc.vector.tensor_scalar(s, v, 1.0, None, Alu.is_ge)
                for oh in range(2):
                    nc.sync.dma_start(out=out_v[sl, oh], in_=s[:, oh, :])
                if t < 3:
                    if RESET_VARIANT[bl] == "smask":
                        # v = (s == 0) * v : in0 is the u8 spike mask
                        nc.vector.scalar_tensor_tensor(
                            v, s, 0.0, v, Alu.is_equal, Alu.mult
                        )
                    else:
                        vnew = vp.tile([P, 2, N], F32, name="v")
                        nc.vector.scalar_tensor_tensor(
                            vnew, v, 1.0, v, Alu.is_lt, Alu.mult
                        )
                        vs[bl] = vnew

    if LEGALIZE:
        _legalize_waits(nc)
    return nc


_nc_cache = None


def _get_nc():
    global _nc_cache
    if _nc_cache is None:
        _nc_cache = _build()
    return _nc_cache


def _tb_index(core, sl):
    bl, t = sl // T, sl % T
    return t * B + core * B_LOC + bl


def _prep_core_inputs(x, core):
    """Per-core input prep: gather slices, transpose to c-major with the
    n -> (q, r) -> j = 256r + q permutation, split fp16 hi/lo + bf16 hi."""
    import ml_dtypes

    idx = [_tb_index(core, sl) for sl in range(SL)]
    xc = x[idx]                                  # (SL, N, C) f32
    xct = xc.transpose(0, 2, 1)                  # (SL, C, N)
    xct = xct.reshape(SL, C, 256, 4).transpose(0, 1, 3, 2).reshape(SL, C, N)
    xh = xct.astype(np.float16)
    xl = (xct - xh.astype(np.float32)).astype(np.float16)
    xhb = xh.astype(ml_dtypes.bfloat16)
    return (np.ascontiguousarray(xh), np.ascontiguousarray(xl),
            np.ascontiguousarray(xhb))


def kernel(x, W, gamma, beta, _trace=False, _trace_kwargs=None):
    import ml_dtypes
    from concourse.bass_utils import run_bass_kernel_spmd

    x = np.ascontiguousarray(np.asarray(x, dtype=np.float32))
    W = np.ascontiguousarray(np.asarray(W, dtype=np.float32))
    gamma = np.ascontiguousarray(np.asarray(gamma, dtype=np.float32))
    beta = np.ascontiguousarray(np.asarray(beta, dtype=np.float32))

    Wh = W.astype(np.float16)
    Wl = (W - Wh.astype(np.float32)).astype(np.float16)
    whT = np.ascontiguousarray((Wh * np.float16(0.5)).T)
    wlT = np.ascontiguousarray(
        (Wl.astype(np.float32) * 0.5).astype(ml_dtypes.bfloat16).T
    )

    nc = _get_nc()
    in_maps = []
    for k in range(N_CORES):
        xh, xl, xhb = _prep_core_inputs(x, k)
        in_maps.append({
            "xh": xh, "xl": xl, "xhb": xhb,
            "whT": whT, "wlT": wlT, "gamma": gamma, "beta": beta,
        })
    kwargs = dict(_trace_kwargs or {})
    res = run_bass_kernel_spmd(
        nc, in_maps, core_ids=list(range(N_CORES)), trace=_trace, **kwargs
    )
    out = np.empty((T * B, N, C), dtype=np.float32)
    for k in range(N_CORES):
        ok = res.results[k]["out"]
        for sl in range(SL):
            out[_tb_index(k, sl)] = ok[sl].reshape(N, C).astype(np.float32)
    if _trace:
        return out, res
    return out
